# revision 31
# baseline (speedup 1.0000x reference)
"""Trainium2 Bass kernel for BlankEmbedding (embedding lookup + blank shift-accumulate).

Reference semantics:
    out = emb[x]                                    # [B, S, D] gather
    preblank[p] = (x[p+1]==BLANK) & (x[p]!=BLANK)   (per row; zero-padded shifts)
    out[p+k] += preblank[p] * emb[x[p]]  for k in 1..3

Strategy: data-parallel over the 16384 flattened tokens, 2048 per core.
The device gathers int8-quantized rows (global absmax/127 scale; ~7.8e-3
rel err vs the 2e-2 budget) and stores them unmodified; the host applies
the scale while unsharding. Sparse blank fixups (P(blank)=1/50257) are
recomputed on-device in int16 and placed by the host.

- Gathers run on the SWDGE indirect-DMA path: descgen is the bottleneck
  (~1.1us per 128-row instruction, engine-serial; measured that neither
  multiple SWDGE queues nor InstDMAGatherAnt beat it once its ~9us mlp
  ucode library load is accounted). Layout ix[p, j] = token 16p + j, so
  each partition holds 16 consecutive tokens and each store descriptor
  is contiguous in DRAM.
- int8 end-to-end halves both the random-row reads (1KB rows) and the
  store traffic vs the bf16 variant, and removes the DVE dequant stage.
- The two fixup gathers sit right after the first main gather so their
  adds + fixout store complete under the main chain instead of tailing
  it. Unused fixup slots read the appended zero row (index VOCAB).
"""

import numpy as np

VOCAB = 50257
ZROW = VOCAB                 # appended all-zeros table row (no-op addend)
DIM = 1024
BLANK = 100
N_BLANKS = 3
B, S = 4, 4096
N_CORES = 8
TOK = B * S                  # 16384 flattened tokens
TPC = TOK // N_CORES         # 2048 tokens per core
P = 128                      # SBUF partitions
NJ = TPC // P                # 16 tokens per partition

_CACHE = {}


KFIX = 16


def _build_nc():
    from concourse import bacc, mybir, tile
    import concourse.bass as bass

    nc = bacc.Bacc(
        "TRN2", target_bir_lowering=False, debug=False, num_devices=1
    )
    i8 = mybir.dt.int8
    i16 = mybir.dt.int16
    i32 = mybir.dt.int32

    ix_dram = nc.dram_tensor("ix", [P, NJ], i32, kind="ExternalInput")
    emb8 = nc.dram_tensor("emb8", [VOCAB + 1, DIM], i8, kind="ExternalInput")
    fix_dram = nc.dram_tensor("fix", [P, 1], i32, kind="ExternalInput")
    out = nc.dram_tensor("out", [TPC, DIM], i8, kind="ExternalOutput")
    fixout = nc.dram_tensor("fixout", [KFIX, DIM], i16, kind="ExternalOutput")

    with tile.TileContext(nc) as tc:
        with tc.tile_pool(name="sbuf", bufs=1) as pool:
            ix_all = pool.tile([P, NJ], i32)
            fix_sb = pool.tile([P, 1], i32)
            # ix on gpsimd's own SWDGE queue: ~1us descgen right after the
            # entry barrier beats the cross-engine HWDGE latency (~2.9us)
            nc.gpsimd.dma_start(out=ix_all[:], in_=ix_dram[:])
            nc.scalar.dma_start(out=fix_sb[:], in_=fix_dram[:])

            g8 = pool.tile([P, NJ * DIM], i8)
            out3 = out[:].rearrange("(p j) d -> p j d", p=P, j=NJ)

            def main_gather(j):
                nc.gpsimd.indirect_dma_start(
                    out=g8[:, j * DIM : (j + 1) * DIM],
                    out_offset=None,
                    in_=emb8[:],
                    in_offset=bass.IndirectOffsetOnAxis(
                        ap=ix_all[:, j : j + 1], axis=0
                    ),
                )
                nc.sync.dma_start(
                    out=out3[:, j : j + 1, :],
                    in_=g8[:, j * DIM : (j + 1) * DIM],
                )

            for j in range(NJ):
                main_gather(j)
            # single fixup gather ends the descgen chain: its small tail
            # (reads + int16 adds + fixout store, ~1.4us) hides under the
            # main chain's read drain. Slot k's xt/s1/s2 addends sit at
            # partitions k / 32+k / 64+k (32-aligned groups for the
            # SBUF-copy realignment below); unused slots read the ZROW row
            fx = pool.tile([P, DIM], i8)
            nc.gpsimd.indirect_dma_start(
                out=fx[:80, :], out_offset=None, in_=emb8[:],
                in_offset=bass.IndirectOffsetOnAxis(
                    ap=fix_sb[:80, 0:1], axis=0
                ),
            )

            # fixout[k] = emb8[xt_k] + emb8[s1_k] + emb8[s2_k] in int16
            w0 = pool.tile([P, DIM], i16)
            nc.vector.tensor_scalar(
                out=w0[:80, :], in0=fx[:80, :],
                scalar1=1.0, scalar2=None, op0=mybir.AluOpType.mult,
            )
            g1 = pool.tile([P, DIM], i16)
            g2 = pool.tile([P, DIM], i16)
            nc.scalar.dma_start(out=g1[0:KFIX, :], in_=w0[32 : 32 + KFIX, :])
            nc.scalar.dma_start(out=g2[0:KFIX, :], in_=w0[64 : 64 + KFIX, :])
            nc.vector.tensor_tensor(
                out=g1[0:KFIX, :], in0=g1[0:KFIX, :], in1=g2[0:KFIX, :],
                op=mybir.AluOpType.add,
            )
            nc.vector.tensor_tensor(
                out=w0[0:KFIX, :], in0=w0[0:KFIX, :], in1=g1[0:KFIX, :],
                op=mybir.AluOpType.add,
            )
            nc.scalar.dma_start(out=fixout[:], in_=w0[:KFIX, :])

    nc.compile()
    return nc


def get_nc():
    if "nc" not in _CACHE:
        _CACHE["nc"] = _build_nc()
    return _CACHE["nc"]


def _corrections(x2):
    """Exact reference semantics: list of (global_target_row, src_token)."""
    is_blank = x2 == BLANK
    prev = np.zeros_like(is_blank)
    prev[:, 1:] = is_blank[:, :-1]
    first_blank = is_blank & ~prev
    out = []
    for b, f in np.argwhere(first_blank):
        if f == 0:
            continue  # run at row start: reference shifts in zeros
        p = f - 1
        src_tok = int(x2[b, p])
        for k in range(1, N_BLANKS + 1):
            s = p + k
            if s >= S:
                break
            out.append((b * S + s, src_tok))
    return out


def shard_inputs(x, emb_table):
    """Returns (in_maps, fix_targets, kfix, has2, scale); fix_targets[c]
    maps fixout slot -> core-local target row."""
    x2 = np.asarray(x).astype(np.int64).reshape(B, S)
    flat = x2.reshape(-1).astype(np.int32)
    emb_f = np.asarray(emb_table, dtype=np.float32)
    scale = float(np.abs(emb_f).max()) / 127.0
    emb_i8 = np.vstack(
        [
            np.clip(np.rint(emb_f / scale), -127, 127).astype(np.int8),
            np.zeros((1, DIM), dtype=np.int8),
        ]
    )

    # per-target slots: tgt -> up to 2 src tokens (two blank runs can land
    # on one target only at distance 2; adjacent first-blanks are impossible)
    per_tgt = {}
    for tgt, src in _corrections(x2):
        per_tgt.setdefault(tgt, []).append(src)
    assert all(len(v) <= 2 for v in per_tgt.values()), per_tgt

    in_maps = []
    fix_targets = []
    for c in range(N_CORES):
        base = c * TPC
        ix = np.ascontiguousarray(flat[base : base + TPC].reshape(P, NJ))

        # slot k: xt at partition k, s1 at 32+k, s2 at 64+k; ZROW elsewhere
        fix = np.full((P, 1), ZROW, dtype=np.int32)
        mine = {t: v for t, v in per_tgt.items() if base <= t < base + TPC}
        assert len(mine) <= KFIX, "fixup slot overflow"
        targets = {}
        for slot, (t, srcs) in enumerate(mine.items()):
            fix[slot, 0] = flat[t]
            fix[32 + slot, 0] = srcs[0]
            if len(srcs) > 1:
                fix[64 + slot, 0] = srcs[1]
            targets[slot] = t - base
        fix_targets.append(targets)
        in_maps.append({"ix": ix, "emb8": emb_i8, "fix": fix})
    return in_maps, fix_targets, scale


def assemble_output(results, fix_targets, scale):
    parts = []
    for c in range(N_CORES):
        part = results[c]["out"].astype(np.float32) * scale
        targets = fix_targets[c]
        if targets:
            fo = results[c]["fixout"]
            for slot, loc in targets.items():
                part[loc] = fo[slot].astype(np.float32) * scale
        parts.append(part)
    return np.concatenate(parts, axis=0).reshape(B, S, DIM)


def kernel(x, emb_table):
    from concourse.bass_utils import run_bass_kernel_spmd

    in_maps, fix_targets, scale = shard_inputs(x, emb_table)
    nc = get_nc()
    res = run_bass_kernel_spmd(nc, in_maps, core_ids=list(range(N_CORES)))
    return assemble_output(res.results, fix_targets, scale)
